# revision 42
# baseline (speedup 1.0000x reference)
"""Causal attention (with faithful missing-head-transpose reshape bug) on 8 Trainium2 cores.

Problem: B=2, T=2048, E=1024, H=16, dk=64.
  qkv = x @ w_qkv.T ; q,k,v split; per-head causal softmax attention;
  out = att_out[B,H,T,dk].reshape(B,T,E)  (NO head transpose — faithful bug);
  y = out @ w_proj.T + b_proj

Key observation: because of the missing transpose, output rows
y[b, 128h : 128h+128, :] depend ONLY on head h.  Sharding (batch x head-group)
over 8 cores therefore needs NO collectives: core c handles batch c//4 and
heads 4*(c%4) .. 4*(c%4)+3, producing output rows [512g, 512g+512) of batch b.

Per-core kernel (bf16 matmuls, fp32 PSUM accumulation):
  - x arrives as four contiguous [E, 512] column-window slabs so the QKV
    matmuls can start as soon as the first slab chunk lands
  - QKV is FUSED into the attention window loop: the Q/K/V psum groups for
    window w+1 are emitted between the score/PV groups of window w, acting
    as useful PE filler while the scalar engine catches up on exps (the
    scalar engine is the per-jc rate limiter during attention)
  - scores computed TRANSPOSED: S^T[j,i] (keys on partitions) so exp(S^T)
    is directly the lhsT-ready P^T for the P@V matmul
  - V carries a ones-column per head: row 64 of each P@V psum is the
    softmax denominator for free
  - causal handling: fully-masked 128x128 blocks of diagonal windows are
    never computed (score matmul N is trimmed); the per-block triangle is
    masked with one [128,128] mask multiply; trimmed es columns live in
    dedicated pre-zeroed tiles so the PV matmul can read the full window
  - normalization: denominator row rides along in the praw drain copy,
    spread across lanes by DMA, reciprocal on DVE, broadcast back via a
    K=1 outer-product matmul
  - the buggy reshape is free in row-major DRAM; the projection's lhsT
    (R^T chunks) is read from a [dup, shift-by-one] att2 buffer with a
    stride-16 access pattern
"""

import os
import sys

import numpy as np

for _p in ("/opt/trn_rl_repo", "/root/.axon_site/_ro/trn_rl_repo"):
    if os.path.isdir(_p) and _p not in sys.path:
        sys.path.insert(0, _p)

import ml_dtypes  # noqa: E402

import concourse.bacc as bacc  # noqa: E402
import concourse.mybir as mybir  # noqa: E402
from concourse.bass import ds, ts  # noqa: E402
from concourse.tile import TileContext  # noqa: E402

F32 = mybir.dt.float32
BF16 = mybir.dt.bfloat16
AF = mybir.ActivationFunctionType
BF16NP = ml_dtypes.bfloat16

P = 128
E = 1024
DK = 64
HPC = 4  # heads per core
TW = 512  # i-window for scores / pv matmuls
EC = E // P  # 8 e-chunks
DC = (HPC * DK) // P  # 2 chunks of per-core qk features
FW = E // 512  # 2 output-feature windows


def build_nc(T=2048):
    W = T // TW  # i-windows (4)
    JPW = TW // P  # j-chunks per window (4)
    TC = T // P  # t-chunks for V (16)
    RR = (T * DK) // E  # rows of R per head (128)
    TT = E // DK  # 16 t-positions per R row
    NSP = HPC * TW // P  # denom elems per lane after spread (16)

    nc = bacc.Bacc("TRN2", target_bir_lowering=False, debug=False)
    # host-packed layouts: partition-major so every DMA line is contiguous
    # (multi-KB) — [p, e, d] means SBUF partition p gets chunk e's row p
    xws = [
        nc.declare_dram_parameter(f"xw{w}", [P, EC, TW], BF16, isOutput=False)
        for w in range(W)
    ]
    wqT = nc.declare_dram_parameter("wqT", [P, EC, HPC * DK], BF16, isOutput=False)
    wkT = nc.declare_dram_parameter("wkT", [P, EC, HPC * DK], BF16, isOutput=False)
    wvT = nc.declare_dram_parameter("wvT", [P, EC, HPC * DK], BF16, isOutput=False)
    wpT = nc.declare_dram_parameter("wpT", [P, EC, E], BF16, isOutput=False)
    # bias pre-replicated across partitions: fused into the ysb drain add
    bpr = nc.declare_dram_parameter("bpr", [P, E], F32, isOutput=False)
    y = nc.declare_dram_parameter("y", [HPC * RR, E], F32, isOutput=True)

    with nc.allow_low_precision(reason="bf16 matmuls; accumulation stays fp32 in PSUM"), TileContext(nc) as tc:
        with (
            tc.tile_pool(name="const", bufs=1) as const,
            tc.tile_pool(name="qkvout", bufs=1) as qkv_pool,
            tc.tile_pool(name="wp", bufs=1) as wp_pool,
            tc.tile_pool(name="xin", bufs=1) as xpool,
            tc.tile_pool(name="wqkv", bufs=1) as wq_pool,
        ):
            ones = const.tile([P, P], BF16)
            nc.vector.memset(ones, 1.0)
            zer = const.tile([P, P], BF16)
            nc.vector.memset(zer, 0.0)
            wsrc = const.tile([P, TW], BF16)
            nc.vector.memset(wsrc, 0.0)
            bp_sb = const.tile([P, E], F32)
            # lower-triangle [128,128] mask: keep j' <= i' within the
            # diagonal block (shared by every diagonal offset)
            tri = const.tile([P, P], BF16, name="tri", tag="tri")
            nc.vector.memset(tri, 1.0)
            nc.gpsimd.affine_select(
                out=tri,
                in_=tri,
                pattern=[[1, P]],
                compare_op=mybir.AluOpType.is_ge,
                fill=0.0,
                base=0,
                channel_multiplier=-1,
            )

            wq_sb = wq_pool.tile([P, EC, HPC * DK], BF16)
            wk_sb = wq_pool.tile([P, EC, HPC * DK], BF16)
            wv_sb = wq_pool.tile([P, EC, HPC * DK], BF16)
            wp_sb = wp_pool.tile([P, EC, E], BF16)
            xp = [xpool.tile([P, EC, TW], BF16, name=f"xp{w}") for w in range(W)]

            qT = qkv_pool.tile([P, DC, T], BF16)
            kT = qkv_pool.tile([P, DC, T], BF16)
            vsb = qkv_pool.tile([P, TC, HPC * (DK + 1)], BF16)
            # ones column per head (row 64 of each P@V psum = softmax denominator)
            nc.vector.memset(
                vsb.rearrange("p t (h c) -> p t h c", c=DK + 1)[:, :, :, DK : DK + 1], 1.0
            )

            # ---- input DMAs: ~256KB pieces, priority order, 3 queues ----
            # early work (wq/wk/xw0 feed the first matmuls) is split fine so
            # the first piece lands fast on every queue
            engs3 = [nc.sync, nc.gpsimd, nc.scalar]
            qi = 0

            def dma(out, in_, late=False):
                nonlocal qi
                # late transfers stay off the scalar queue (it runs exps)
                pool_n = 2 if late else 3
                engs3[qi % pool_n].dma_start(out=out, in_=in_)
                qi += 1

            # fine interleave of wq / xw0 / wk so the e-major QK matmuls can
            # chase the data chunk by chunk; rotate the queue offset each
            # round so no single kind lands on one queue
            for e2 in range(4):
                sl = ds(2 * e2, 2)
                dma(wq_sb[:, sl, :], wqT[:, sl, :])
                dma(xp[0][:, sl, :], xws[0][:, sl, :])
                dma(wk_sb[:, sl, :], wkT[:, sl, :])
                qi += 1
                if e2 == 1:
                    dma(wv_sb[:, 0:4, :], wvT[:, 0:4, :])
                if e2 == 2:
                    dma(wv_sb[:, 4:8, :], wvT[:, 4:8, :])
            for h in range(4):
                dma(xp[1][:, 2 * h : 2 * h + 2, :], xws[1][:, 2 * h : 2 * h + 2, :])
            # late: remaining x windows + w_proj off the scalar queue
            for w in range(2, W):
                dma(xp[w][:, 0:4, :], xws[w][:, 0:4, :], late=True)
                dma(xp[w][:, 4:8, :], xws[w][:, 4:8, :], late=True)
            dma(wp_sb[:, 0:4, :], wpT[:, 0:4, :], late=True)
            dma(wp_sb[:, 4:8, :], wpT[:, 4:8, :], late=True)
            dma(bp_sb, bpr[:, :], late=True)

            with (
                tc.tile_pool(name="att", bufs=1) as att_pool,
                tc.tile_pool(name="des", bufs=1) as des_pool,
                tc.tile_pool(name="exps", bufs=6) as epool,
                tc.tile_pool(name="rec", bufs=2) as rpool,
                tc.tile_pool(name="psa", bufs=1, space="PSUM") as psa,
                tc.tile_pool(name="yout", bufs=2) as ypool,
            ):
                att2 = []
                for h in range(HPC):
                    a = att_pool.tile([P, T], BF16, name=f"att2_{h}", tag=f"att2_{h}")
                    att2.append(a)
                    # last col of shifted half never written; keep sim happy
                    nc.vector.memset(a[DK : 2 * DK, T - 1 : T], 0.0)

                # dedicated diagonal es tiles: trimmed (fully-masked) columns
                # are zeroed once and never written again, so PV can read the
                # full window width
                des = {}
                for qq in range(JPW):
                    for p in range(2):
                        t_ = des_pool.tile(
                            [P, 2 * TW], BF16, name=f"des{qq}_{p}", tag=f"des{qq}_{p}"
                        )
                        # only the trimmed (fully-masked) columns must be zero
                        for s in range(2):
                            if qq > 0:
                                nc.vector.memset(t_[:, ds(TW * s, P * qq)], 0.0)
                        des[(qq, p)] = t_

                # ---------- QKV group emitters (psum borrowed from tag "s") ----------
                # gpsimd cannot touch PSUM: drains go on DVE, or on the
                # scalar engine (activation-Copy) when it has slack
                def drain_copy(out, in_):
                    nc.vector.tensor_copy(out, in_)

                def scalar_copy(out, in_):
                    nc.scalar.activation(out, in_, AF.Copy)

                def emit_qk_group(dst, wsb, dc, w):
                    ps = psa.tile([P, 2 * TW], F32, tag="s", bufs=2, name="ps_qk")
                    for e in range(EC):
                        nc.tensor.matmul(
                            ps[:, 0:TW],
                            wsb[:, e, ts(dc, P)],
                            xp[w][:, e, :],
                            start=(e == 0),
                            stop=(e == EC - 1),
                        )
                    drain_copy(dst[:, dc, ds(TW * w, TW)], ps[:, 0:TW])

                def emit_v_group(t):
                    ps = psa.tile([P, 2 * TW], F32, tag="s", bufs=2, name="ps_v")
                    for e in range(EC):
                        nc.tensor.matmul(
                            ps[:, 0 : HPC * DK],
                            xp[t // JPW][:, e, ts(t % JPW, P)],
                            wv_sb[:, e, :],
                            start=(e == 0),
                            stop=(e == EC - 1),
                        )
                    drain_copy(
                        vsb.rearrange("p t (h c) -> p t h c", c=DK + 1)[:, t, :, 0:DK],
                        ps[:, 0 : HPC * DK].rearrange("p (h d) -> p h d", d=DK),
                    )

                # ---------- window 0 QKV runs standalone (DMA-paced) ----------
                # e-major across 4 concurrent Q/K groups (borrowing the idle
                # pv psum banks) so each arriving x chunk is consumed at once
                qk0 = {}
                for i, (dst, wsb, dc) in enumerate(
                    ((qT, wq_sb, 0), (qT, wq_sb, 1), (kT, wk_sb, 0), (kT, wk_sb, 1))
                ):
                    qk0[i] = psa.tile([P, TW], F32, tag=f"pv{i}", bufs=1, name="qk0")
                for e in range(EC):
                    for i, (dst, wsb, dc) in enumerate(
                        ((qT, wq_sb, 0), (qT, wq_sb, 1), (kT, wk_sb, 0), (kT, wk_sb, 1))
                    ):
                        nc.tensor.matmul(
                            qk0[i],
                            wsb[:, e, ts(dc, P)],
                            xp[0][:, e, :],
                            start=(e == 0),
                            stop=(e == EC - 1),
                        )
                for i, (dst, wsb, dc) in enumerate(
                    ((qT, wq_sb, 0), (qT, wq_sb, 1), (kT, wk_sb, 0), (kT, wk_sb, 1))
                ):
                    drain_copy(dst[:, dc, 0:TW], qk0[i])

                # ---------- fused attention + QKV pipeline ----------
                norm2 = None

                def _emit_pv(pvs, ess, jc, njc, trim):
                    # diagonal blocks: the first `trim` columns of es are zero
                    # by construction, so skip them (their psum region already
                    # holds the final accumulation from earlier jc)
                    for p in range(2):
                        es = ess[p]
                        for sub in range(2):
                            h = 2 * p + sub
                            nc.tensor.matmul(
                                pvs[h][0 : DK + 1, ds(trim, TW - trim)],
                                vsb[:, jc, ds((DK + 1) * h, DK + 1)],
                                es[:, ds(TW * sub + trim, TW - trim)],
                                start=(jc == 0),
                                stop=(jc == njc - 1),
                            )

                for w in range(W):
                    njc = JPW * (w + 1)
                    # PE filler schedule: iteration -> list of thunks.
                    # K(w) / V(diag of w) land inside window w itself;
                    # Q(w+1) is emitted early in window w.
                    fillers = {}

                    def add_fill(i, fn):
                        fillers.setdefault(i, []).append(fn)

                    # spread the filler groups EVENLY across the window: the
                    # exp deficit (~0.35us/jc) accrues at every jc, so bunched
                    # fillers leave mid-window PE gaps.  Constraints: K(w)
                    # before iteration 4w, V(t) before iteration t+2, Q(w+1)
                    # anywhere.
                    # Q(w+1) is handled at the END of the window (see below):
                    # emitting it early would block the in-order PE queue on
                    # its x-window DMA.  Interior fillers: K(w) before its
                    # first diagonal st, V(t) two iterations ahead of its pv.
                    if w == 0:
                        add_fill(1, lambda: emit_v_group(0))
                        add_fill(2, lambda: emit_v_group(1))
                        add_fill(3, lambda: emit_v_group(2))
                        add_fill(3, lambda: emit_v_group(3))
                    elif w == 1:
                        add_fill(1, lambda w=w: emit_qk_group(kT, wk_sb, 0, w))
                        add_fill(2, lambda w=w: emit_qk_group(kT, wk_sb, 1, w))
                        for qq in range(JPW):
                            add_fill(JPW + qq, lambda t=JPW + qq: emit_v_group(t))
                    elif w == 2:
                        add_fill(2, lambda w=w: emit_qk_group(kT, wk_sb, 0, w))
                        add_fill(5, lambda w=w: emit_qk_group(kT, wk_sb, 1, w))
                        for qq in range(JPW):
                            add_fill(8 + qq, lambda t=8 + qq: emit_v_group(t))
                    else:
                        add_fill(4, lambda w=w: emit_qk_group(kT, wk_sb, 0, w))
                        add_fill(8, lambda w=w: emit_qk_group(kT, wk_sb, 1, w))
                        for qq in range(JPW):
                            add_fill(12 + qq, lambda t=12 + qq: emit_v_group(t))

                    pvs = [
                        psa.tile([P, TW], F32, tag=f"pv{h}", bufs=1, name=f"pv{h}")
                        for h in range(HPC)
                    ]
                    pend = []
                    pv_open = False
                    for jc in range(njc):
                        qq = jc - JPW * w  # >=0 on causal-diagonal blocks
                        trim = P * qq if qq >= 0 else 0
                        ess = []
                        for p in range(2):
                            st = psa.tile([P, 2 * TW], F32, tag="s", bufs=2, name="st")
                            for sub in range(2):
                                nc.tensor.matmul(
                                    st[:, ds(TW * sub + trim, TW - trim)],
                                    kT[ds(DK * sub, DK), p, ts(jc, P)],
                                    qT[ds(DK * sub, DK), p, ds(TW * w + trim, TW - trim)],
                                    start=True,
                                    stop=True,
                                )
                            if qq >= 0:
                                es = des[(qq, p)]
                                for sub in range(2):
                                    sl = ds(TW * sub + trim, TW - trim)
                                    nc.scalar.activation(es[:, sl], st[:, sl], AF.Exp, scale=1.0 / 8.0)
                                for sub in range(2):
                                    blk = ds(TW * sub + trim, P)
                                    nc.vector.tensor_mul(es[:, blk], es[:, blk], tri)
                            else:
                                es = epool.tile([P, 2 * TW], BF16, name="es")
                                nc.scalar.activation(es, st, AF.Exp, scale=1.0 / 8.0)
                            ess.append(es)
                        pend.append((ess, jc))
                        # fillers go after the score matmuls so exp starts ASAP
                        fns = fillers.get(jc, ())
                        for fn in fns:
                            fn()
                        if not fns and pv_open and jc < njc - 1:
                            # no real filler here: two zero-matmuls into an
                            # open pv group absorb the per-jc exp deficit so
                            # the PE never idles (idling costs a 3us p-state
                            # ramp on top of the gap itself)
                            for k in range(2):
                                nc.tensor.matmul(
                                    pvs[(jc + k) % HPC][0 : DK + 1, :],
                                    zer[:, 0 : DK + 1],
                                    wsrc,
                                    start=False,
                                    stop=False,
                                )
                        if len(pend) > 2:
                            e0 = pend.pop(0)
                            tq = e0[1] - JPW * w
                            _emit_pv(pvs, e0[0], e0[1], njc, P * tq if tq > 0 else 0)
                            pv_open = True
                        if jc == 3 and norm2 is not None:
                            norm2()
                            norm2 = None
                    # Q(w+1) lands here: useful PE work covering the last
                    # exps this window is still waiting on
                    if w < W - 1:
                        emit_qk_group(qT, wq_sb, 0, w + 1)
                        emit_qk_group(qT, wq_sb, 1, w + 1)
                    for e0 in pend:
                        tq = e0[1] - JPW * w
                        _emit_pv(pvs, e0[0], e0[1], njc, P * tq if tq > 0 else 0)

                    # ---- drain pv banks; denom row rides along in row 64 ----
                    praw = rpool.tile([P, HPC * TW], BF16, name="praw", tag="praw", bufs=2)
                    for h in range(HPC):
                        # only after the LAST window is scalar truly done with
                        # exps — mid-run its copies would delay the next window
                        cp = scalar_copy if (w == W - 1 and h % 2 == 1) else drain_copy
                        cp(praw[0 : DK + 1, ds(TW * h, TW)], pvs[h][0 : DK + 1, :])

                    def _norm2(w=w, praw=praw, proj=None):
                        # spread denom row across lanes (split over 2 queues),
                        # reciprocal, spread back
                        sp = rpool.tile([P, NSP], BF16, name="sp", tag="sp")
                        hs = HPC * TW // 2
                        src = praw[DK : DK + 1, :].rearrange("a (p c) -> a p c", c=NSP)
                        nc.sync.dma_start(out=sp[0 : P // 2, 0:NSP], in_=src[:, 0 : P // 2, :])
                        nc.gpsimd.dma_start(out=sp[P // 2 : P, 0:NSP], in_=src[:, P // 2 : P, :])
                        rec = rpool.tile([P, NSP], F32, name="rec", tag="rec")
                        nc.vector.reciprocal(out=rec, in_=sp[:, 0:NSP])
                        spb = rpool.tile([P, NSP], BF16, name="spb", tag="spb")
                        nc.vector.tensor_copy(spb, rec)
                        recb = rpool.tile([P, HPC * TW], BF16, name="recb", tag="recb")
                        dst = recb[DK : DK + 1, :].rearrange("a (p c) -> a p c", c=NSP)
                        nc.sync.dma_start(out=dst[:, 0 : P // 2, :], in_=spb[0 : P // 2, 0:NSP])
                        nc.gpsimd.dma_start(out=dst[:, P // 2 : P, :], in_=spb[P // 2 : P, 0:NSP])
                        # all heads' rt/mul/shift first so the shift-DMA
                        # latencies overlap each other (and the projections)
                        for h in range(HPC):
                            rt = psa.tile([P, 2 * TW], F32, tag="s", bufs=2, name="rt")
                            nc.tensor.matmul(
                                rt[0:DK, 0:TW],
                                ones[DK : DK + 1, 0:DK],
                                recb[DK : DK + 1, ds(TW * h, TW)],
                                start=True,
                                stop=True,
                            )
                            nc.vector.tensor_mul(
                                att2[h][0:DK, ds(TW * w, TW)],
                                rt[0:DK, 0:TW],
                                praw[0:DK, ds(TW * h, TW)],
                            )
                            if w == 0:
                                nc.sync.dma_start(
                                    out=att2[h][DK : 2 * DK, 0 : TW - 1],
                                    in_=att2[h][0:DK, 1:TW],
                                )
                            else:
                                nc.sync.dma_start(
                                    out=att2[h][DK : 2 * DK, TW * w - 1 : TW * (w + 1) - 1],
                                    in_=att2[h][0:DK, ds(TW * w, TW)],
                                )
                        if proj is not None:
                            # short warm chain: covers the first head's
                            # mul+shift latency before its projection
                            wt2 = psa.tile([P, 2 * TW], F32, tag="s", bufs=2, name="wt2")
                            for i in range(10):
                                nc.tensor.matmul(
                                    wt2[0 : DK + 1, 0:TW],
                                    zer[:, 0 : DK + 1],
                                    wsrc,
                                    start=(i == 0),
                                    stop=(i == 9),
                                )
                            for h in range(HPC):
                                proj(h)

                    norm2 = _norm2

                # cover the last window's normalization-chain latency with a
                # PE warm chain, then run it with the per-head projection fused in
                def _proj(h):
                    a2v = att2[h].rearrange("p (r t) -> p r t", t=TT)
                    for fw in range(FW):
                        # rotate across all 4 pv banks so no group waits on
                        # the previous group's drain
                        yp = psa.tile(
                            [P, TW], F32, tag=f"pv{(2 * h + fw) % HPC}", bufs=1, name="yp"
                        )
                        for m in range(EC):
                            nc.tensor.matmul(
                                yp[0:RR, :],
                                a2v[:, :, 2 * m : 2 * m + 1],
                                wp_sb[:, m, ds(512 * fw, 512)],
                                start=(m == 0),
                                stop=(m == EC - 1),
                            )
                        ysb = ypool.tile([P, 512], F32, name="ysb")
                        # bias add fused into the drain (same DVE cost as copy)
                        nc.vector.tensor_add(
                            ysb[0:RR, :], yp[0:RR, :], bp_sb[0:RR, ds(512 * fw, 512)]
                        )
                        # y output goes on gpsimd/scalar queues: sync carries
                        # the shift DMAs the next head is waiting on
                        yeng = nc.gpsimd if (2 * h + fw) % 2 == 0 else nc.scalar
                        yeng.dma_start(
                            out=y[ds(RR * h, RR), ds(512 * fw, 512)], in_=ysb[0:RR, :]
                        )

                wtf = psa.tile([P, 2 * TW], F32, tag="s", bufs=2, name="wtf")
                NWARMF = 14
                for i in range(NWARMF):
                    nc.tensor.matmul(
                        wtf[0 : DK + 1, 0:TW],
                        zer[:, 0 : DK + 1],
                        wsrc,
                        start=(i == 0),
                        stop=(i == NWARMF - 1),
                    )
                norm2(proj=_proj)
    nc.compile()
    return nc


_CACHE = {}
LAST_RESULT = None


def _get_nc(T=2048):
    key = ("nc", T)
    if key not in _CACHE:
        _CACHE[key] = build_nc(T=T)
    return _CACHE[key]


def _pack(a2d):
    """[E, D] -> partition-major [P, EC, D] (chunk e's row p lands on
    SBUF partition p), contiguous per partition."""
    Erows, D = a2d.shape
    return np.ascontiguousarray(
        a2d.reshape(Erows // P, P, D).transpose(1, 0, 2)
    )


def make_in_maps(x, w_qkv, w_proj, b_proj):
    B, T, _E = x.shape
    W = T // TW
    in_maps = []
    wpTh = _pack(w_proj.T.astype(BF16NP))
    bph = np.ascontiguousarray(
        np.broadcast_to(b_proj.astype(np.float32).reshape(1, E), (P, E))
    )
    xTs = [x[b].T.astype(BF16NP) for b in range(B)]
    xwin = [
        [_pack(xTs[b][:, TW * w : TW * (w + 1)]) for w in range(W)]
        for b in range(B)
    ]
    for c in range(8):
        b, g = divmod(c, 4)
        r0 = HPC * DK * g  # 256*g
        sl = slice(r0, r0 + HPC * DK)
        m = {
            "wqT": _pack(w_qkv[sl, :].T.astype(BF16NP)),
            "wkT": _pack(w_qkv[E:][sl, :].T.astype(BF16NP)),
            "wvT": _pack(w_qkv[2 * E :][sl, :].T.astype(BF16NP)),
            "wpT": wpTh,
            "bpr": bph,
        }
        for w in range(W):
            m[f"xw{w}"] = xwin[b][w]
        in_maps.append(m)
    return in_maps


def kernel(x, w_qkv, w_proj, b_proj):
    global LAST_RESULT
    from concourse.bass_utils import run_bass_kernel_spmd

    x = np.asarray(x, dtype=np.float32)
    w_qkv = np.asarray(w_qkv, dtype=np.float32)
    w_proj = np.asarray(w_proj, dtype=np.float32)
    b_proj = np.asarray(b_proj, dtype=np.float32)
    B, T, _E = x.shape

    nc = _get_nc(T=T)
    in_maps = make_in_maps(x, w_qkv, w_proj, b_proj)
    res = run_bass_kernel_spmd(nc, in_maps, core_ids=list(range(8)))
    LAST_RESULT = res

    out = np.empty((B, T, E), dtype=np.float32)
    rows = HPC * ((T * DK) // E)  # 512 rows per core
    for c in range(8):
        b, g = divmod(c, 4)
        out[b, rows * g : rows * (g + 1), :] = res.results[c]["y"]
    return out


# revision 44
# speedup vs baseline: 1.1748x; 1.1748x over previous
"""Causal attention (with faithful missing-head-transpose reshape bug) on 8 Trainium2 cores.

Problem: B=2, T=2048, E=1024, H=16, dk=64.
  qkv = x @ w_qkv.T ; q,k,v split; per-head causal softmax attention;
  out = att_out[B,H,T,dk].reshape(B,T,E)  (NO head transpose — faithful bug);
  y = out @ w_proj.T + b_proj

Key observation: because of the missing transpose, output rows
y[b, 128h : 128h+128, :] depend ONLY on head h.  Sharding (batch x head-group)
over 8 cores therefore needs NO collectives: core c handles batch c//4 and
heads 4*(c%4) .. 4*(c%4)+3, producing output rows [512g, 512g+512) of batch b.

Per-core kernel (bf16 matmuls, fp32 PSUM accumulation):
  - x arrives as four contiguous [E, 512] column-window slabs so the QKV
    matmuls can start as soon as the first slab chunk lands
  - QKV is FUSED into the attention window loop: the Q/K/V psum groups for
    window w+1 are emitted between the score/PV groups of window w, acting
    as useful PE filler while the scalar engine catches up on exps (the
    scalar engine is the per-jc rate limiter during attention)
  - scores computed TRANSPOSED: S^T[j,i] (keys on partitions) so exp(S^T)
    is directly the lhsT-ready P^T for the P@V matmul
  - V carries a ones-column per head: row 64 of each P@V psum is the
    softmax denominator for free
  - causal handling: fully-masked 128x128 blocks of diagonal windows are
    never computed (score matmul N is trimmed); the per-block triangle is
    masked with one [128,128] mask multiply; trimmed es columns live in
    dedicated pre-zeroed tiles so the PV matmul can read the full window
  - normalization: denominator row rides along in the praw drain copy,
    spread across lanes by DMA, reciprocal on DVE, broadcast back via a
    K=1 outer-product matmul
  - the buggy reshape is free in row-major DRAM; the projection's lhsT
    (R^T chunks) is read from a [dup, shift-by-one] att2 buffer with a
    stride-16 access pattern
"""

import os
import sys

import numpy as np

for _p in ("/opt/trn_rl_repo", "/root/.axon_site/_ro/trn_rl_repo"):
    if os.path.isdir(_p) and _p not in sys.path:
        sys.path.insert(0, _p)

import ml_dtypes  # noqa: E402

import concourse.bacc as bacc  # noqa: E402
import concourse.mybir as mybir  # noqa: E402
from concourse.bass import ds, ts  # noqa: E402
from concourse.tile import TileContext  # noqa: E402

F32 = mybir.dt.float32
BF16 = mybir.dt.bfloat16
AF = mybir.ActivationFunctionType
BF16NP = ml_dtypes.bfloat16

P = 128
E = 1024
DK = 64
HPC = 4  # heads per core
TW = 512  # i-window for scores / pv matmuls
EC = E // P  # 8 e-chunks
DC = (HPC * DK) // P  # 2 chunks of per-core qk features
FW = E // 512  # 2 output-feature windows


def build_nc(T=2048):
    W = T // TW  # i-windows (4)
    JPW = TW // P  # j-chunks per window (4)
    TC = T // P  # t-chunks for V (16)
    RR = (T * DK) // E  # rows of R per head (128)
    TT = E // DK  # 16 t-positions per R row
    NSP = HPC * TW // P  # denom elems per lane after spread (16)

    nc = bacc.Bacc("TRN2", target_bir_lowering=False, debug=False)
    # host-packed layouts: partition-major so every DMA line is contiguous
    # (multi-KB) — [p, e, d] means SBUF partition p gets chunk e's row p
    xws = [
        nc.declare_dram_parameter(f"xw{w}", [P, EC, TW], BF16, isOutput=False)
        for w in range(W)
    ]
    wqT = nc.declare_dram_parameter("wqT", [P, EC, HPC * DK], BF16, isOutput=False)
    wkT = nc.declare_dram_parameter("wkT", [P, EC, HPC * DK], BF16, isOutput=False)
    wvT = nc.declare_dram_parameter("wvT", [P, EC, HPC * DK], BF16, isOutput=False)
    wpT = nc.declare_dram_parameter("wpT", [P, EC, E], BF16, isOutput=False)
    # bias pre-replicated across partitions: fused into the ysb drain add
    bpr = nc.declare_dram_parameter("bpr", [P, E], F32, isOutput=False)
    y = nc.declare_dram_parameter("y", [HPC * RR, E], F32, isOutput=True)

    with nc.allow_low_precision(reason="bf16 matmuls; accumulation stays fp32 in PSUM"), TileContext(nc) as tc:
        with (
            tc.tile_pool(name="const", bufs=1) as const,
            tc.tile_pool(name="qkvout", bufs=1) as qkv_pool,
            tc.tile_pool(name="wp", bufs=1) as wp_pool,
            tc.tile_pool(name="xin", bufs=1) as xpool,
            tc.tile_pool(name="wqkv", bufs=1) as wq_pool,
        ):
            ones = const.tile([P, P], BF16)
            nc.vector.memset(ones, 1.0)
            zer = const.tile([P, P], BF16)
            nc.vector.memset(zer, 0.0)
            wsrc = const.tile([P, TW], BF16)
            nc.vector.memset(wsrc, 0.0)
            bp_sb = const.tile([P, E], F32)
            # lower-triangle [128,128] mask: keep j' <= i' within the
            # diagonal block (shared by every diagonal offset)
            tri = const.tile([P, P], BF16, name="tri", tag="tri")
            nc.vector.memset(tri, 1.0)
            nc.gpsimd.affine_select(
                out=tri,
                in_=tri,
                pattern=[[1, P]],
                compare_op=mybir.AluOpType.is_ge,
                fill=0.0,
                base=0,
                channel_multiplier=-1,
            )

            wq_sb = wq_pool.tile([P, EC, HPC * DK], BF16)
            wk_sb = wq_pool.tile([P, EC, HPC * DK], BF16)
            wv_sb = wq_pool.tile([P, EC, HPC * DK], BF16)
            wp_sb = wp_pool.tile([P, EC, E], BF16)
            xp = [xpool.tile([P, EC, TW], BF16, name=f"xp{w}") for w in range(W)]

            qT = qkv_pool.tile([P, DC, T], BF16)
            kT = qkv_pool.tile([P, DC, T], BF16)
            vsb = qkv_pool.tile([P, TC, HPC * (DK + 1)], BF16)
            # ones column per head (row 64 of each P@V psum = softmax denominator)
            nc.vector.memset(
                vsb.rearrange("p t (h c) -> p t h c", c=DK + 1)[:, :, :, DK : DK + 1], 1.0
            )

            # ---- input DMAs: ~256KB pieces, priority order, 3 queues ----
            # early work (wq/wk/xw0 feed the first matmuls) is split fine so
            # the first piece lands fast on every queue
            engs3 = [nc.sync, nc.gpsimd, nc.scalar]
            qi = 0

            def dma(out, in_, late=False):
                nonlocal qi
                # late transfers stay off the scalar queue (it runs exps)
                pool_n = 2 if late else 3
                engs3[qi % pool_n].dma_start(out=out, in_=in_)
                qi += 1

            # fine interleave of wq / xw0 / wk so the e-major QK matmuls can
            # chase the data chunk by chunk; rotate the queue offset each
            # round so no single kind lands on one queue
            for e2 in range(4):
                sl = ds(2 * e2, 2)
                dma(wq_sb[:, sl, :], wqT[:, sl, :])
                dma(xp[0][:, sl, :], xws[0][:, sl, :])
                dma(wk_sb[:, sl, :], wkT[:, sl, :])
                qi += 1
                if e2 == 1:
                    dma(wv_sb[:, 0:4, :], wvT[:, 0:4, :])
                if e2 == 2:
                    dma(wv_sb[:, 4:8, :], wvT[:, 4:8, :])
            for h in range(4):
                dma(xp[1][:, 2 * h : 2 * h + 2, :], xws[1][:, 2 * h : 2 * h + 2, :])
            # late: remaining x windows + w_proj off the scalar queue
            for w in range(2, W):
                dma(xp[w][:, 0:4, :], xws[w][:, 0:4, :], late=True)
                dma(xp[w][:, 4:8, :], xws[w][:, 4:8, :], late=True)
            dma(wp_sb[:, 0:4, :], wpT[:, 0:4, :], late=True)
            dma(wp_sb[:, 4:8, :], wpT[:, 4:8, :], late=True)
            dma(bp_sb, bpr[:, :], late=True)

            with (
                tc.tile_pool(name="att", bufs=1) as att_pool,
                tc.tile_pool(name="des", bufs=1) as des_pool,
                tc.tile_pool(name="exps", bufs=6) as epool,
                tc.tile_pool(name="rec", bufs=2) as rpool,
                tc.tile_pool(name="psa", bufs=1, space="PSUM") as psa,
                tc.tile_pool(name="yout", bufs=2) as ypool,
            ):
                att2 = []
                for h in range(HPC):
                    a = att_pool.tile([P, T], BF16, name=f"att2_{h}", tag=f"att2_{h}")
                    att2.append(a)
                    # last col of shifted half never written; keep sim happy
                    nc.vector.memset(a[DK : 2 * DK, T - 1 : T], 0.0)

                # dedicated diagonal es tiles: trimmed (fully-masked) columns
                # are zeroed once and never written again, so PV can read the
                # full window width
                des = {}
                for qq in range(JPW):
                    for p in range(2):
                        t_ = des_pool.tile(
                            [P, 2 * TW], BF16, name=f"des{qq}_{p}", tag=f"des{qq}_{p}"
                        )
                        # only the trimmed (fully-masked) columns must be zero
                        for s in range(2):
                            if qq > 0:
                                nc.vector.memset(t_[:, ds(TW * s, P * qq)], 0.0)
                        des[(qq, p)] = t_

                # ---------- QKV group emitters (psum borrowed from tag "s") ----------
                # gpsimd cannot touch PSUM: drains go on DVE, or on the
                # scalar engine (activation-Copy) when it has slack
                def drain_copy(out, in_):
                    nc.vector.tensor_copy(out, in_)

                def scalar_copy(out, in_):
                    nc.scalar.activation(out, in_, AF.Copy)

                def emit_qk_group(dst, wsb, dc, w):
                    ps = psa.tile([P, 2 * TW], F32, tag="s", bufs=2, name="ps_qk")
                    for e in range(EC):
                        nc.tensor.matmul(
                            ps[:, 0:TW],
                            wsb[:, e, ts(dc, P)],
                            xp[w][:, e, :],
                            start=(e == 0),
                            stop=(e == EC - 1),
                        )
                    drain_copy(dst[:, dc, ds(TW * w, TW)], ps[:, 0:TW])

                def emit_v_group(t):
                    ps = psa.tile([P, 2 * TW], F32, tag="s", bufs=2, name="ps_v")
                    for e in range(EC):
                        nc.tensor.matmul(
                            ps[:, 0 : HPC * DK],
                            xp[t // JPW][:, e, ts(t % JPW, P)],
                            wv_sb[:, e, :],
                            start=(e == 0),
                            stop=(e == EC - 1),
                        )
                    drain_copy(
                        vsb.rearrange("p t (h c) -> p t h c", c=DK + 1)[:, t, :, 0:DK],
                        ps[:, 0 : HPC * DK].rearrange("p (h d) -> p h d", d=DK),
                    )

                # ---------- window 0 QKV runs standalone (DMA-paced) ----------
                # e-major across 4 concurrent Q/K groups (borrowing the idle
                # pv psum banks) so each arriving x chunk is consumed at once
                qk0 = {}
                for i, (dst, wsb, dc) in enumerate(
                    ((qT, wq_sb, 0), (qT, wq_sb, 1), (kT, wk_sb, 0), (kT, wk_sb, 1))
                ):
                    qk0[i] = psa.tile([P, TW], F32, tag=f"pv{i}", bufs=1, name="qk0")
                for e in range(EC):
                    for i, (dst, wsb, dc) in enumerate(
                        ((qT, wq_sb, 0), (qT, wq_sb, 1), (kT, wk_sb, 0), (kT, wk_sb, 1))
                    ):
                        nc.tensor.matmul(
                            qk0[i],
                            wsb[:, e, ts(dc, P)],
                            xp[0][:, e, :],
                            start=(e == 0),
                            stop=(e == EC - 1),
                        )
                for i, (dst, wsb, dc) in enumerate(
                    ((qT, wq_sb, 0), (qT, wq_sb, 1), (kT, wk_sb, 0), (kT, wk_sb, 1))
                ):
                    drain_copy(dst[:, dc, 0:TW], qk0[i])

                # ---------- fused attention + QKV pipeline ----------
                norm2 = None

                def _emit_pv(pvs, ess, jc, njc, trim):
                    # diagonal blocks: the first `trim` columns of es are zero
                    # by construction, so skip them (their psum region already
                    # holds the final accumulation from earlier jc)
                    for p in range(2):
                        es = ess[p]
                        for sub in range(2):
                            h = 2 * p + sub
                            nc.tensor.matmul(
                                pvs[h][0 : DK + 1, ds(trim, TW - trim)],
                                vsb[:, jc, ds((DK + 1) * h, DK + 1)],
                                es[:, ds(TW * sub + trim, TW - trim)],
                                start=(jc == 0),
                                stop=(jc == njc - 1),
                            )

                for w in range(W):
                    njc = JPW * (w + 1)
                    # PE filler schedule: iteration -> list of thunks.
                    # K(w) / V(diag of w) land inside window w itself;
                    # Q(w+1) is emitted early in window w.
                    fillers = {}

                    def add_fill(i, fn):
                        fillers.setdefault(i, []).append(fn)

                    # spread the filler groups EVENLY across the window: the
                    # exp deficit (~0.35us/jc) accrues at every jc, so bunched
                    # fillers leave mid-window PE gaps.  Constraints: K(w)
                    # before iteration 4w, V(t) before iteration t+2, Q(w+1)
                    # anywhere.
                    # Q(w+1) is handled at the END of the window (see below):
                    # emitting it early would block the in-order PE queue on
                    # its x-window DMA.  Interior fillers: K(w) before its
                    # first diagonal st, V(t) two iterations ahead of its pv.
                    if w == 0:
                        add_fill(1, lambda: emit_v_group(0))
                        add_fill(2, lambda: emit_v_group(1))
                        add_fill(3, lambda: emit_v_group(2))
                        add_fill(3, lambda: emit_v_group(3))
                    elif w == 1:
                        add_fill(1, lambda w=w: emit_qk_group(kT, wk_sb, 0, w))
                        add_fill(2, lambda w=w: emit_qk_group(kT, wk_sb, 1, w))
                        for qq in range(JPW):
                            add_fill(JPW + qq, lambda t=JPW + qq: emit_v_group(t))
                    elif w == 2:
                        add_fill(2, lambda w=w: emit_qk_group(kT, wk_sb, 0, w))
                        add_fill(5, lambda w=w: emit_qk_group(kT, wk_sb, 1, w))
                        for qq in range(JPW):
                            add_fill(8 + qq, lambda t=8 + qq: emit_v_group(t))
                    else:
                        add_fill(4, lambda w=w: emit_qk_group(kT, wk_sb, 0, w))
                        add_fill(8, lambda w=w: emit_qk_group(kT, wk_sb, 1, w))
                        for qq in range(JPW):
                            add_fill(12 + qq, lambda t=12 + qq: emit_v_group(t))

                    pvs = [
                        psa.tile([P, TW], F32, tag=f"pv{h}", bufs=1, name=f"pv{h}")
                        for h in range(HPC)
                    ]
                    pend = []
                    for jc in range(njc):
                        qq = jc - JPW * w  # >=0 on causal-diagonal blocks
                        trim = P * qq if qq >= 0 else 0
                        ess = []
                        for p in range(2):
                            st = psa.tile([P, 2 * TW], F32, tag="s", bufs=2, name="st")
                            for sub in range(2):
                                nc.tensor.matmul(
                                    st[:, ds(TW * sub + trim, TW - trim)],
                                    kT[ds(DK * sub, DK), p, ts(jc, P)],
                                    qT[ds(DK * sub, DK), p, ds(TW * w + trim, TW - trim)],
                                    start=True,
                                    stop=True,
                                )
                            if qq >= 0:
                                es = des[(qq, p)]
                                for sub in range(2):
                                    sl = ds(TW * sub + trim, TW - trim)
                                    nc.scalar.activation(es[:, sl], st[:, sl], AF.Exp, scale=1.0 / 8.0)
                                for sub in range(2):
                                    blk = ds(TW * sub + trim, P)
                                    nc.vector.tensor_mul(es[:, blk], es[:, blk], tri)
                            else:
                                es = epool.tile([P, 2 * TW], BF16, name="es")
                                nc.scalar.activation(es, st, AF.Exp, scale=1.0 / 8.0)
                            ess.append(es)
                        pend.append((ess, jc))
                        # fillers go after the score matmuls so exp starts ASAP
                        for fn in fillers.get(jc, ()):
                            fn()
                        if len(pend) > 2:
                            e0 = pend.pop(0)
                            tq = e0[1] - JPW * w
                            _emit_pv(pvs, e0[0], e0[1], njc, P * tq if tq > 0 else 0)
                        if jc == 3 and norm2 is not None:
                            norm2()
                            norm2 = None
                    # Q(w+1) lands here: useful PE work covering the last
                    # exps this window is still waiting on
                    if w < W - 1:
                        emit_qk_group(qT, wq_sb, 0, w + 1)
                        emit_qk_group(qT, wq_sb, 1, w + 1)
                    for e0 in pend:
                        tq = e0[1] - JPW * w
                        _emit_pv(pvs, e0[0], e0[1], njc, P * tq if tq > 0 else 0)

                    # ---- drain pv banks; denom row rides along in row 64 ----
                    praw = rpool.tile([P, HPC * TW], BF16, name="praw", tag="praw", bufs=2)
                    for h in range(HPC):
                        # only after the LAST window is scalar truly done with
                        # exps — mid-run its copies would delay the next window
                        cp = scalar_copy if (w == W - 1 and h % 2 == 1) else drain_copy
                        cp(praw[0 : DK + 1, ds(TW * h, TW)], pvs[h][0 : DK + 1, :])

                    def _norm2(w=w, praw=praw, proj=None):
                        # spread denom row across lanes (split over 2 queues),
                        # reciprocal, spread back
                        sp = rpool.tile([P, NSP], BF16, name="sp", tag="sp")
                        hs = HPC * TW // 2
                        src = praw[DK : DK + 1, :].rearrange("a (p c) -> a p c", c=NSP)
                        nc.sync.dma_start(out=sp[0 : P // 2, 0:NSP], in_=src[:, 0 : P // 2, :])
                        nc.gpsimd.dma_start(out=sp[P // 2 : P, 0:NSP], in_=src[:, P // 2 : P, :])
                        rec = rpool.tile([P, NSP], F32, name="rec", tag="rec")
                        nc.vector.reciprocal(out=rec, in_=sp[:, 0:NSP])
                        spb = rpool.tile([P, NSP], BF16, name="spb", tag="spb")
                        nc.vector.tensor_copy(spb, rec)
                        recb = rpool.tile([P, HPC * TW], BF16, name="recb", tag="recb")
                        dst = recb[DK : DK + 1, :].rearrange("a (p c) -> a p c", c=NSP)
                        nc.sync.dma_start(out=dst[:, 0 : P // 2, :], in_=spb[0 : P // 2, 0:NSP])
                        nc.gpsimd.dma_start(out=dst[:, P // 2 : P, :], in_=spb[P // 2 : P, 0:NSP])
                        # all heads' rt/mul/shift first so the shift-DMA
                        # latencies overlap each other (and the projections)
                        for h in range(HPC):
                            rt = psa.tile([P, 2 * TW], F32, tag="s", bufs=2, name="rt")
                            nc.tensor.matmul(
                                rt[0:DK, 0:TW],
                                ones[DK : DK + 1, 0:DK],
                                recb[DK : DK + 1, ds(TW * h, TW)],
                                start=True,
                                stop=True,
                            )
                            nc.vector.tensor_mul(
                                att2[h][0:DK, ds(TW * w, TW)],
                                rt[0:DK, 0:TW],
                                praw[0:DK, ds(TW * h, TW)],
                            )
                            if w == 0:
                                nc.sync.dma_start(
                                    out=att2[h][DK : 2 * DK, 0 : TW - 1],
                                    in_=att2[h][0:DK, 1:TW],
                                )
                            else:
                                nc.sync.dma_start(
                                    out=att2[h][DK : 2 * DK, TW * w - 1 : TW * (w + 1) - 1],
                                    in_=att2[h][0:DK, ds(TW * w, TW)],
                                )
                        if proj is not None:
                            # short warm chain: covers the first head's
                            # mul+shift latency before its projection
                            wt2 = psa.tile([P, 2 * TW], F32, tag="s", bufs=2, name="wt2")
                            for i in range(10):
                                nc.tensor.matmul(
                                    wt2[0 : DK + 1, 0:TW],
                                    zer[:, 0 : DK + 1],
                                    wsrc,
                                    start=(i == 0),
                                    stop=(i == 9),
                                )
                            for h in range(HPC):
                                proj(h)

                    norm2 = _norm2

                # cover the last window's normalization-chain latency with a
                # PE warm chain, then run it with the per-head projection fused in
                def _proj(h):
                    a2v = att2[h].rearrange("p (r t) -> p r t", t=TT)
                    for fw in range(FW):
                        # rotate across all 4 pv banks so no group waits on
                        # the previous group's drain
                        yp = psa.tile(
                            [P, TW], F32, tag=f"pv{(2 * h + fw) % HPC}", bufs=1, name="yp"
                        )
                        for m in range(EC):
                            nc.tensor.matmul(
                                yp[0:RR, :],
                                a2v[:, :, 2 * m : 2 * m + 1],
                                wp_sb[:, m, ds(512 * fw, 512)],
                                start=(m == 0),
                                stop=(m == EC - 1),
                            )
                        ysb = ypool.tile([P, 512], F32, name="ysb")
                        # bias add fused into the drain (same DVE cost as copy)
                        nc.vector.tensor_add(
                            ysb[0:RR, :], yp[0:RR, :], bp_sb[0:RR, ds(512 * fw, 512)]
                        )
                        # y output goes on gpsimd/scalar queues: sync carries
                        # the shift DMAs the next head is waiting on
                        yeng = nc.gpsimd if (2 * h + fw) % 2 == 0 else nc.scalar
                        yeng.dma_start(
                            out=y[ds(RR * h, RR), ds(512 * fw, 512)], in_=ysb[0:RR, :]
                        )

                wtf = psa.tile([P, 2 * TW], F32, tag="s", bufs=2, name="wtf")
                NWARMF = 14
                for i in range(NWARMF):
                    nc.tensor.matmul(
                        wtf[0 : DK + 1, 0:TW],
                        zer[:, 0 : DK + 1],
                        wsrc,
                        start=(i == 0),
                        stop=(i == NWARMF - 1),
                    )
                norm2(proj=_proj)
    nc.compile()
    return nc


_CACHE = {}
LAST_RESULT = None


def _get_nc(T=2048):
    key = ("nc", T)
    if key not in _CACHE:
        _CACHE[key] = build_nc(T=T)
    return _CACHE[key]


def _pack(a2d):
    """[E, D] -> partition-major [P, EC, D] (chunk e's row p lands on
    SBUF partition p), contiguous per partition."""
    Erows, D = a2d.shape
    return np.ascontiguousarray(
        a2d.reshape(Erows // P, P, D).transpose(1, 0, 2)
    )


def make_in_maps(x, w_qkv, w_proj, b_proj):
    B, T, _E = x.shape
    W = T // TW
    in_maps = []
    wpTh = _pack(w_proj.T.astype(BF16NP))
    bph = np.ascontiguousarray(
        np.broadcast_to(b_proj.astype(np.float32).reshape(1, E), (P, E))
    )
    xTs = [x[b].T.astype(BF16NP) for b in range(B)]
    xwin = [
        [_pack(xTs[b][:, TW * w : TW * (w + 1)]) for w in range(W)]
        for b in range(B)
    ]
    for c in range(8):
        b, g = divmod(c, 4)
        r0 = HPC * DK * g  # 256*g
        sl = slice(r0, r0 + HPC * DK)
        m = {
            "wqT": _pack(w_qkv[sl, :].T.astype(BF16NP)),
            "wkT": _pack(w_qkv[E:][sl, :].T.astype(BF16NP)),
            "wvT": _pack(w_qkv[2 * E :][sl, :].T.astype(BF16NP)),
            "wpT": wpTh,
            "bpr": bph,
        }
        for w in range(W):
            m[f"xw{w}"] = xwin[b][w]
        in_maps.append(m)
    return in_maps


def kernel(x, w_qkv, w_proj, b_proj):
    global LAST_RESULT
    from concourse.bass_utils import run_bass_kernel_spmd

    x = np.asarray(x, dtype=np.float32)
    w_qkv = np.asarray(w_qkv, dtype=np.float32)
    w_proj = np.asarray(w_proj, dtype=np.float32)
    b_proj = np.asarray(b_proj, dtype=np.float32)
    B, T, _E = x.shape

    nc = _get_nc(T=T)
    in_maps = make_in_maps(x, w_qkv, w_proj, b_proj)
    res = run_bass_kernel_spmd(nc, in_maps, core_ids=list(range(8)))
    LAST_RESULT = res

    out = np.empty((B, T, E), dtype=np.float32)
    rows = HPC * ((T * DK) // E)  # 512 rows per core
    for c in range(8):
        b, g = divmod(c, 4)
        out[b, rows * g : rows * (g + 1), :] = res.results[c]["y"]
    return out
